# revision 1
# baseline (speedup 1.0000x reference)
"""CommNet forward kernel for 8 Trainium2 NeuronCores.

Reference computation (per sample of N=32 agents, batch B=16384):
    h   = relu(obs @ enc_w + enc_b)                    # [B,N,64]
    2x:  msg = (sum_n h - h)/31
         h   = relu(concat(h, msg) @ comm_w[r] + comm_b[r])
    hid = relu(h @ out_w1 + out_b1)
    q   = hid @ out_w2 + out_b2; q[avail==0] = -1e10

Device strategy (pure data parallel, batch split 8 ways):
  * activations feature-major [feat(part), row(free)]; four 512-row
    groups per 2048-row super-iteration, packed 2x2 into the PE array
    via tile_position (K=64, M=64 quadrants) so all 16 subarrays
    compute concurrently.  Groups at (p-half, f-half) positions
    (0,1)/(1,0) swap every matmul layer; 4 permuting layers = identity,
    so the out2 layout matches the obs layout.
  * comm round rewritten as h @ W_self + S @ W_sum with
    W_self = W_h - W_m/31, W_sum = W_m/31, S = per-sample agent sum.
    S comes from identity-weight matmuls with a step-0 (broadcast)
    output AP that accumulates the 32 agent columns of each sample into
    one PSUM column; the S @ W_sum term re-broadcasts S via a step-0
    rhs AP into the same accumulation group as the W_self matmul.
  * relu+bias fused into the PSUM->SBUF evacuation (DVE dual-op
    tensor_scalar for enc/out1, ScalarE activation for the rounds)
  * mask+final bias folded host-side into pen = where(avail, out_b2, -1e10);
    pen is added on the PE (identity-lhsT matmul accumulate) and the q
    bank evacuated with a ScalarE copy
  * host pre-packs obs into the feature-major layout and unpacks q
    (layout work is free on host; the device does all the FLOPs)
"""

import contextlib
import sys

import numpy as np

sys.path.insert(0, "/opt/trn_rl_repo")

import ml_dtypes  # noqa: E402

B, N, OBS, H, A, NR = 16384, 32, 64, 64, 16, 2
NCORES = 8
RPC = B * N // NCORES   # rows per core = 65536

SUP = 2048              # rows per super-iteration (4 groups of 512)
GRP = 512               # rows per group (one fp32 PSUM bank)
NSUP = RPC // SUP
NS_G = GRP // N         # samples per group = 16
NS_H = 2 * NS_G         # samples per partition-half per super = 32

_cache = {}


def _build_device_program():
    import concourse.bacc as bacc
    import concourse.mybir as mybir
    from concourse import tile

    F32 = mybir.dt.float32
    BF16 = mybir.dt.bfloat16

    nc = bacc.Bacc("TRN2", target_bir_lowering=False, debug=False)

    obs_d = nc.dram_tensor("obs_pk", [NSUP, 128, SUP // 2], BF16, kind="ExternalInput")
    pen_d = nc.dram_tensor("pen_pk", [NSUP // 2, 128, GRP], F32, kind="ExternalInput")
    q_d = nc.dram_tensor("q_pk", [NSUP // 2, 128, GRP], BF16, kind="ExternalOutput")

    # replicated-on-both-halves [128, 64] weights; W2 block-diag [128, 32]
    wname = ["Wenc", "Wself0", "Wself1", "Wsum0", "Wsum1", "W1", "idn"]
    w_d = {n: nc.dram_tensor(n, [128, 64], BF16, kind="ExternalInput") for n in wname}
    w_d["W2"] = nc.dram_tensor("W2", [128, 32], BF16, kind="ExternalInput")
    w_d["idnq"] = nc.dram_tensor("idnq", [128, 32], F32, kind="ExternalInput")
    bname = ["be", "b0", "b1", "bh"]
    b_d = {n: nc.dram_tensor(n, [128, 1], F32, kind="ExternalInput") for n in bname}

    FD = GRP
    Relu = mybir.ActivationFunctionType.Relu
    Copy = mybir.ActivationFunctionType.Copy
    ALU = mybir.AluOpType
    QUAD = [(0, 0, 0, 0), (0, 1, 0, 64), (1, 0, 64, 64), (1, 1, 64, 0)]
    # (in p-half, in f-half, rhs part base, out part base); out f-half = in f-half
    # after act: group at (ph, fh) lands at (out_base//64, fh) -> (0,1)/(1,0) swap

    with tile.TileContext(nc) as tc, contextlib.ExitStack() as ctx:
        wp = ctx.enter_context(tc.tile_pool(name="w", bufs=1))
        pool = ctx.enter_context(tc.tile_pool(name="p", bufs=3))
        psum = ctx.enter_context(tc.tile_pool(name="ps", bufs=1, space="PSUM"))

        W = {}
        for n in wname:
            W[n] = wp.tile([128, 64], BF16, tag=n, name=f"w_{n}")
            nc.sync.dma_start(W[n][:], w_d[n][:])
        W["W2"] = wp.tile([128, 32], BF16, tag="W2", name="w_W2")
        nc.sync.dma_start(W["W2"][:], w_d["W2"][:])
        W["idnq"] = wp.tile([128, 32], F32, tag="idnq", name="w_idnq")
        nc.sync.dma_start(W["idnq"][:], w_d["idnq"][:])
        BIAS = {}
        for n in bname:
            BIAS[n] = wp.tile([128, 1], F32, tag=n, name=f"b_{n}")
            nc.sync.dma_start(BIAS[n][:], b_d[n][:])

        def layer_mms(ps, wt, rhs_t):
            """4 concurrent K=64/M=64 matmuls (one per group) into ps[128,1024]."""
            for ph, fh, rb, ob in QUAD:
                nc.tensor.matmul(
                    ps[ob:ob + 64, fh * FD:(fh + 1) * FD],
                    wt[rb:rb + 64, :],
                    rhs_t[ph * 64:(ph + 1) * 64, fh * FD:(fh + 1) * FD],
                    start=True, stop=True, tile_position=(rb, ob),
                )

        for s in range(NSUP):
            obs_t = pool.tile([128, 2 * FD], BF16, tag="obs")
            nc.sync.dma_start(obs_t[:], obs_d[s])

            psE = psum.tile([128, 2 * FD], F32, tag="stg", bufs=3)
            layer_mms(psE, W["Wenc"], obs_t)
            h = pool.tile([128, 2 * FD], BF16, tag="h0")
            nc.vector.tensor_scalar(h[:], psE[:], BIAS["be"][:], 0.0,
                                    ALU.add, ALU.max)

            for r in range(NR):
                psS = psum.tile([128, NS_H], F32, tag="S")
                for hp, tp in ((0, 0), (64, 64)):
                    for sh in range(2):
                        rhs = h[hp:hp + 64, sh * FD:(sh + 1) * FD] \
                            .rearrange("p (S n) -> p n S", n=N)
                        outS = psS[hp:hp + 64, sh * NS_G:(sh + 1) * NS_G] \
                            .unsqueeze(1).broadcast_to([64, N, NS_G])
                        nc.tensor.matmul(outS, W["idn"][hp:hp + 64, :], rhs,
                                         start=True, stop=True,
                                         tile_position=(tp, tp))
                S2 = pool.tile([128, NS_H], BF16, tag="S2")
                nc.vector.tensor_copy(S2[:], psS[:])

                psR = psum.tile([128, 2 * FD], F32, tag="stg", bufs=3)
                for ph, fh, rb, ob in QUAD:
                    nc.tensor.matmul(
                        psR[ob:ob + 64, fh * FD:(fh + 1) * FD],
                        W[f"Wself{r}"][rb:rb + 64, :],
                        h[ph * 64:(ph + 1) * 64, fh * FD:(fh + 1) * FD],
                        start=True, stop=False, tile_position=(rb, ob),
                    )
                    sb = S2[ph * 64:(ph + 1) * 64, fh * NS_G:(fh + 1) * NS_G] \
                        .unsqueeze(2).broadcast_to([64, NS_G, N])
                    nc.tensor.matmul(
                        psR[ob:ob + 64, fh * FD:(fh + 1) * FD],
                        W[f"Wsum{r}"][rb:rb + 64, :], sb,
                        start=False, stop=True, tile_position=(rb, ob),
                    )
                h = pool.tile([128, 2 * FD], BF16, tag=f"h{1 + r}")
                nc.scalar.activation(h[:], psR[:], Relu, bias=BIAS[f"b{r}"][:])

            psH = psum.tile([128, 2 * FD], F32, tag="stg", bufs=3)
            layer_mms(psH, W["W1"], h)
            hid = pool.tile([128, 2 * FD], BF16, tag="hid")
            nc.vector.tensor_scalar(hid[:], psH[:], BIAS["bh"][:], 0.0,
                                    ALU.add, ALU.max)

            # out2: block-diag over partition pairs; two col positions.
            # q banks of even/odd super-iters pack into one [128, FD] bank
            # (partition halves) so the evacuation runs full-width half as often.
            k = s % 2
            qo = 64 * k
            if k == 0:
                pen_t = pool.tile([128, FD], F32, tag="pen")
                nc.sync.dma_start(pen_t[:], pen_d[s // 2])
                psQ = psum.tile([128, FD], F32, tag="q")
                pers = (pen_t, psQ)
            else:
                pen_t, psQ = pers
            nc.tensor.matmul(psQ[qo:qo + 32, :], W["W2"][:], hid[:, 0:FD],
                             start=True, stop=False, tile_position=(0, qo),
                             skip_group_check=True)
            nc.tensor.matmul(psQ[qo:qo + 32, :], W["idnq"][qo:qo + 32, :],
                             pen_t[qo:qo + 32, :],
                             start=False, stop=True, tile_position=(qo % 128 // 32 * 32, qo),
                             skip_group_check=True)
            nc.tensor.matmul(psQ[qo + 32:qo + 64, :], W["W2"][:], hid[:, FD:2 * FD],
                             start=True, stop=False, tile_position=(0, qo + 32),
                             skip_group_check=True)
            nc.tensor.matmul(psQ[qo + 32:qo + 64, :], W["idnq"][qo + 32:qo + 64, :],
                             pen_t[qo + 32:qo + 64, :],
                             start=False, stop=True,
                             tile_position=((qo + 32) % 128 // 32 * 32, qo + 32),
                             skip_group_check=True)
            if k == 1:
                q_sb = pool.tile([128, FD], BF16, tag="qsb")
                nc.scalar.activation(q_sb[:], psQ[:], Copy)
                nc.sync.dma_start(q_d[s // 2], q_sb[:])

    nc.compile()
    return nc


def _prep_host(obs, enc_w, enc_b, comm_w, comm_b, out_w1, out_b1, out_w2, out_b2,
               available_actions):
    """Build per-core input maps (packed layouts + derived weights)."""
    bf16 = ml_dtypes.bfloat16
    f32 = np.float32

    def rep(w):  # replicate [64, m] weight onto both partition halves
        return np.ascontiguousarray(np.concatenate([w, w], axis=0)
                                    .astype(f32)).astype(bf16)

    def bd(w):  # block-diag duplicate [k,m] -> [2k, 2m]
        k, m = w.shape
        o = np.zeros((2 * k, 2 * m), f32)
        o[:k, :m] = w
        o[k:, m:] = w
        return np.ascontiguousarray(o).astype(bf16)

    weights = {"Wenc": rep(enc_w), "W1": rep(out_w1), "W2": bd(out_w2),
               "idn": rep(np.eye(64, dtype=f32)),
               "idnq": np.ascontiguousarray(np.tile(np.eye(32, dtype=f32), (4, 1)))}
    for r in range(NR):
        wh = comm_w[r][:H].astype(f32)
        wm = comm_w[r][H:].astype(f32) / (N - 1)
        weights[f"Wself{r}"] = rep(wh - wm)
        weights[f"Wsum{r}"] = rep(wm)
    biases = {"be": enc_b, "b0": comm_b[0], "b1": comm_b[1], "bh": out_b1}
    biases = {k: np.concatenate([v, v]).astype(f32).reshape(128, 1)
              for k, v in biases.items()}

    rows = np.ascontiguousarray(obs.reshape(B * N, OBS))
    pen = np.where(available_actions.reshape(B * N, A) == 0,
                   f32(-1e10), out_b2.astype(f32)[None, :]).astype(f32)

    in_maps = []
    for c in range(NCORES):
        ro = rows[c * RPC:(c + 1) * RPC]
        # [NSUP, phalf, fhalf, row, feat] -> [NSUP, phalf*feat, fhalf*row]
        opk = ro.reshape(NSUP, 2, 2, GRP, OBS).transpose(0, 1, 4, 2, 3) \
                .reshape(NSUP, 128, SUP // 2).astype(bf16)
        pe = pen[c * RPC:(c + 1) * RPC]
        # q/pen partitions: [fhalf, phalf, action]
        ppk = pe.reshape(NSUP, 2, 2, GRP, A).transpose(0, 2, 1, 4, 3) \
                .reshape(NSUP // 2, 128, GRP).astype(f32)
        m = {"obs_pk": np.ascontiguousarray(opk),
             "pen_pk": np.ascontiguousarray(ppk)}
        m.update(weights)
        m.update(biases)
        in_maps.append(m)
    return in_maps


def _unpack_output(results):
    qs = []
    for r in results:
        qpk = np.asarray(r["q_pk"]).astype(np.float32)  # [NSUP//2, 128, GRP]
        q = qpk.reshape(NSUP, 2, 2, A, GRP).transpose(0, 2, 1, 4, 3) \
               .reshape(RPC, A)
        qs.append(q)
    return np.concatenate(qs, axis=0).reshape(B, N, A)


def run_on_device(in_maps, trace=False):
    from concourse.bass_utils import run_bass_kernel_spmd

    if "nc" not in _cache:
        _cache["nc"] = _build_device_program()
    return run_bass_kernel_spmd(_cache["nc"], in_maps,
                                core_ids=list(range(NCORES)), trace=trace)


def kernel(obs, enc_w, enc_b, comm_w, comm_b, out_w1, out_b1, out_w2, out_b2,
           available_actions):
    args = [np.asarray(x) for x in
            (obs, enc_w, enc_b, comm_w, comm_b, out_w1, out_b1, out_w2, out_b2,
             available_actions)]
    in_maps = _prep_host(*args)
    res = run_on_device(in_maps)
    return _unpack_output(res.results)



# revision 3
# speedup vs baseline: 2.9606x; 2.9606x over previous
"""CommNet forward kernel for 8 Trainium2 NeuronCores.

Reference computation (per sample of N=32 agents, batch B=16384):
    h   = relu(obs @ enc_w + enc_b)                    # [B,N,64]
    2x:  msg = (sum_n h - h)/31
         h   = relu(concat(h, msg) @ comm_w[r] + comm_b[r])
    hid = relu(h @ out_w1 + out_b1)
    q   = hid @ out_w2 + out_b2; q[avail==0] = -1e10

Device strategy (pure data parallel, batch split 8 ways):
  * activations feature-major: chunk = [128 part, 512 col] where
    partitions = 2 row-groups x 64 features, columns = 512 rows.  All
    layers use single K=128/M=128 matmuls with block-diagonal weights
    (two row-groups advance in lockstep), so each layer charges one
    512-row pass through the PE instead of four quadrant passes.
  * comm round rewritten as h @ W_self + S @ W_sum with
    W_self = W_h - W_m/31, W_sum = W_m/31, S = per-sample agent sum.
    S comes from an identity-weight matmul whose step-0 (broadcast)
    output AP accumulates the 32 agent columns of each sample into one
    PSUM column; the S @ W_sum term re-broadcasts S via a step-0 rhs AP
    into the same accumulation group as the W_self matmul.
  * 5-stage modulo software pipeline over 64 chunks per core
    (enc | round0 | round1 | out1 | out2), so the PE always has an
    independent chunk's matmuls to run while an evacuation for another
    chunk is in flight.  Evacuations are split Act/DVE to keep both
    below the PE's per-chunk budget.
  * final bias + action mask applied host-side (exact -1e10 lanes);
    the device emits raw hid @ out_w2 in bf16.
  * host pre-packs obs into the feature-major layout and unpacks q
    (layout work is free on host; the device does all the FLOPs).
"""

import contextlib
import sys

import numpy as np

sys.path.insert(0, "/opt/trn_rl_repo")

import ml_dtypes  # noqa: E402

B, N, OBS, H, A, NR = 16384, 32, 64, 64, 16, 2
NCORES = 8
RPC = B * N // NCORES   # rows per core = 65536

CH = 512                # columns per chunk (rows of one row-group)
NCH = RPC // (2 * CH)   # chunks per core = 64 (each chunk = 2 groups x 512)
NSUP = NCH // 2         # obs DMA granularity: [128, 1024] supers
NS_C = CH // N          # samples per row-group per chunk = 16
NQ = NCH // 4           # q output slabs ([128, 512] = 4 chunks each)

_cache = {}


def _build_device_program():
    import concourse.bacc as bacc
    import concourse.mybir as mybir
    from concourse import tile

    F32 = mybir.dt.float32
    BF16 = mybir.dt.bfloat16

    nc = bacc.Bacc("TRN2", target_bir_lowering=False, debug=False)

    obs_d = nc.dram_tensor("obs_pk", [NSUP, 128, 2 * CH], BF16, kind="ExternalInput")
    q_d = nc.dram_tensor("q_pk", [NQ, 128, CH], BF16, kind="ExternalOutput")

    wname = ["Wenc", "Wself0", "Wself1", "Wsum0", "Wsum1", "W1", "idn"]
    w_d = {n: nc.dram_tensor(n, [128, 128], BF16, kind="ExternalInput") for n in wname}
    w_d["W2"] = nc.dram_tensor("W2", [128, 32], BF16, kind="ExternalInput")
    bname = ["be", "b0", "b1", "bh"]
    b_d = {n: nc.dram_tensor(n, [128, 1], F32, kind="ExternalInput") for n in bname}

    Relu = mybir.ActivationFunctionType.Relu
    Copy = mybir.ActivationFunctionType.Copy
    ALU = mybir.AluOpType

    with tile.TileContext(nc) as tc, contextlib.ExitStack() as ctx:
        wp = ctx.enter_context(tc.tile_pool(name="w", bufs=1))
        pool = ctx.enter_context(tc.tile_pool(name="p", bufs=2))
        psum = ctx.enter_context(tc.tile_pool(name="ps", bufs=1, space="PSUM"))

        W = {}
        for n in wname:
            W[n] = wp.tile([128, 128], BF16, tag=n, name=f"w_{n}")
            nc.sync.dma_start(W[n][:], w_d[n][:])
        W["W2"] = wp.tile([128, 32], BF16, tag="W2", name="w_W2")
        nc.sync.dma_start(W["W2"][:], w_d["W2"][:])
        BIAS = {}
        for n in bname:
            BIAS[n] = wp.tile([128, 1], F32, tag=n, name=f"b_{n}")
            nc.sync.dma_start(BIAS[n][:], b_d[n][:])

        obs_t = {}          # super index -> SBUF tile [128, 1024]
        st = {}             # per-chunk state: h0/h1/h2/hid tiles
        qps = {}            # q PSUM slab per 4-chunk block

        def fetch_super(s):
            if s < NSUP and s not in obs_t:
                obs_t[s] = pool.tile([128, 2 * CH], BF16, tag="obs", bufs=4, name="obs_t")
                nc.sync.dma_start(obs_t[s][:], obs_d[s])

        def obs_slice(c):
            t = obs_t[c // 2]
            f = c % 2
            return t[:, f * CH:(f + 1) * CH]

        # --- pipeline stages (chunk c) -------------------------------------
        def stage_S(c, r, h):
            """S (agent-sum) matmul for round r; returns S2 SBUF tile."""
            psS = psum.tile([128, NS_C], F32, tag="S", bufs=2, name="psS")
            rhs = h[:].rearrange("p (S n) -> p n S", n=N)
            outS = psS[:].unsqueeze(1).broadcast_to([128, N, NS_C])
            nc.tensor.matmul(outS, W["idn"][:], rhs, start=True, stop=True)
            S2 = pool.tile([128, NS_C], BF16, tag=f"S2{r}", bufs=2, name="S2")
            nc.vector.tensor_copy(S2[:], psS[:])
            return S2

        def stage_R(c, r, h, S2):
            """Comm-round linear layer; returns new h tile (evac on Act)."""
            psR = psum.tile([128, CH], F32, tag="stg", bufs=4, name="ps_stg")
            nc.tensor.matmul(psR[:], W[f"Wself{r}"][:], h[:],
                             start=True, stop=False)
            sb = S2[:].unsqueeze(2).broadcast_to([128, NS_C, N])
            nc.tensor.matmul(psR[:], W[f"Wsum{r}"][:], sb,
                             start=False, stop=True)
            hn = pool.tile([128, CH], BF16, tag=f"h{1 + r}", bufs=2, name="hn")
            nc.scalar.activation(hn[:], psR[:], Relu, bias=BIAS[f"b{r}"][:])
            return hn

        def stage0(c):      # encoder
            psE = psum.tile([128, CH], F32, tag="stg", bufs=4, name="ps_stg")
            nc.tensor.matmul(psE[:], W["Wenc"][:], obs_slice(c),
                             start=True, stop=True)
            h0 = pool.tile([128, CH], BF16, tag="h0", bufs=2, name="h0")
            nc.vector.tensor_scalar(h0[:], psE[:], BIAS["be"][:], 0.0,
                                    ALU.add, ALU.max)
            st[c] = {"h0": h0}

        def stage1(c):      # round 0 (S first so the copy hides behind E)
            d = st[c]
            S2 = stage_S(c, 0, d["h0"])
            d["S2"] = S2

        def stage1b(c):
            d = st[c]
            d["h1"] = stage_R(c, 0, d["h0"], d.pop("S2"))
            d.pop("h0")

        def stage2(c):      # round 1
            d = st[c]
            S2 = stage_S(c, 1, d["h1"])
            d["S2"] = S2

        def stage2b(c):
            d = st[c]
            d["h2"] = stage_R(c, 1, d["h1"], d.pop("S2"))
            d.pop("h1")

        def stage3(c):      # out1
            d = st[c]
            psH = psum.tile([128, CH], F32, tag="stg", bufs=4, name="ps_stg")
            nc.tensor.matmul(psH[:], W["W1"][:], d["h2"][:],
                             start=True, stop=True)
            hid = pool.tile([128, CH], BF16, tag="hid", bufs=2, name="hid")
            nc.vector.tensor_scalar(hid[:], psH[:], BIAS["bh"][:], 0.0,
                                    ALU.add, ALU.max)
            d["hid"] = hid
            d.pop("h2")

        def stage4(c):      # out2 into packed q slab
            d = st.pop(c)
            blk, k = c // 4, c % 4
            if k == 0:
                qps[blk] = psum.tile([128, CH], F32, tag="q", bufs=2, name="psQ")
            psQ = qps[blk]
            nc.tensor.matmul(psQ[32 * k:32 * k + 32, :], W["W2"][:],
                             d["hid"][:], start=True, stop=True,
                             tile_position=(0, 32 * k), skip_group_check=True)
            if k == 3:
                q_sb = pool.tile([128, CH], BF16, tag="qsb", bufs=2, name="q_sb")
                nc.scalar.activation(q_sb[:], psQ[:], Copy)
                nc.sync.dma_start(q_d[blk], q_sb[:])
                qps.pop(blk)

        # --- modulo schedule ----------------------------------------------
        fetch_super(0)
        fetch_super(1)
        fetch_super(2)
        # PE issue order per step keeps >=2 independent matmuls between an
        # S matmul and the R matmuls that consume its DVE-copied result.
        for t in range(NCH + 4):
            if t % 2 == 0:
                fetch_super(t // 2 + 3)
            c1, c2 = t - 1, t - 2
            if 0 <= c1 < NCH:
                stage1(c1)
            if 0 <= c2 < NCH:
                stage2(c2)
            if t < NCH:
                stage0(t)
            if 0 <= c1 < NCH:
                stage1b(c1)
            if 0 <= c2 < NCH:
                stage2b(c2)
            if 0 <= t - 3 < NCH:
                stage3(t - 3)
            if 0 <= t - 4 < NCH:
                stage4(t - 4)

    nc.compile()
    return nc


def _prep_host(obs, enc_w, enc_b, comm_w, comm_b, out_w1, out_b1, out_w2, out_b2,
               available_actions):
    """Build per-core input maps (packed layouts + derived weights)."""
    bf16 = ml_dtypes.bfloat16
    f32 = np.float32

    def bd128(w):  # block-diag duplicate [64,64] -> [128,128]
        o = np.zeros((128, 128), f32)
        o[:64, :64] = w
        o[64:, 64:] = w
        return np.ascontiguousarray(o).astype(bf16)

    def bdq(w):  # block-diag duplicate [64,16] -> [128,32]
        o = np.zeros((128, 32), f32)
        o[:64, :16] = w
        o[64:, 16:] = w
        return np.ascontiguousarray(o).astype(bf16)

    weights = {"Wenc": bd128(enc_w.astype(f32)), "W1": bd128(out_w1.astype(f32)),
               "W2": bdq(out_w2.astype(f32)),
               "idn": np.eye(128, dtype=f32).astype(bf16)}
    for r in range(NR):
        wh = comm_w[r][:H].astype(f32)
        wm = comm_w[r][H:].astype(f32) / (N - 1)
        weights[f"Wself{r}"] = bd128(wh - wm)
        weights[f"Wsum{r}"] = bd128(wm)
    biases = {"be": enc_b, "b0": comm_b[0], "b1": comm_b[1], "bh": out_b1}
    biases = {k: np.concatenate([v, v]).astype(f32).reshape(128, 1)
              for k, v in biases.items()}

    rows = np.ascontiguousarray(obs.reshape(B * N, OBS))

    in_maps = []
    for c in range(NCORES):
        ro = rows[c * RPC:(c + 1) * RPC]
        # [NSUP, phalf, fhalf, row, feat] -> [NSUP, phalf*feat, fhalf*row]
        opk = ro.reshape(NSUP, 2, 2, CH, OBS).transpose(0, 1, 4, 2, 3) \
                .reshape(NSUP, 128, 2 * CH).astype(bf16)
        m = {"obs_pk": np.ascontiguousarray(opk)}
        m.update(weights)
        m.update(biases)
        in_maps.append(m)
    return in_maps


def _unpack_output(results, out_b2, available_actions):
    f32 = np.float32
    qs = []
    for r in results:
        qpk = np.asarray(r["q_pk"]).astype(f32)        # [NQ, 128, CH]
        # partitions = (k=chunk%4, p, a); chunk c = 2s + f
        q = qpk.reshape(NQ, 4, 2, A, CH).transpose(0, 1, 2, 4, 3) \
               .reshape(NSUP, 2, 2, CH, A).transpose(0, 2, 1, 3, 4) \
               .reshape(RPC, A)                         # rows: [s, p, f, row]
        qs.append(q)
    q = np.concatenate(qs, axis=0).reshape(B, N, A)
    q = q + out_b2.astype(f32)[None, None, :]
    return np.where(available_actions == 0, f32(-1e10), q)


def run_on_device(in_maps, trace=False):
    from concourse.bass_utils import run_bass_kernel_spmd

    if "nc" not in _cache:
        _cache["nc"] = _build_device_program()
    return run_bass_kernel_spmd(_cache["nc"], in_maps,
                                core_ids=list(range(NCORES)), trace=trace)


def kernel(obs, enc_w, enc_b, comm_w, comm_b, out_w1, out_b1, out_w2, out_b2,
           available_actions):
    args = [np.asarray(x) for x in
            (obs, enc_w, enc_b, comm_w, comm_b, out_w1, out_b1, out_w2, out_b2,
             available_actions)]
    in_maps = _prep_host(*args)
    res = run_on_device(in_maps)
    return _unpack_output(res.results, np.asarray(out_b2),
                          np.asarray(available_actions))


# revision 13
# speedup vs baseline: 3.1241x; 1.0552x over previous
"""CommNet forward kernel for 8 Trainium2 NeuronCores.

Reference computation (per sample of N=32 agents, batch B=16384):
    h   = relu(obs @ enc_w + enc_b)                    # [B,N,64]
    2x:  msg = (sum_n h - h)/31
         h   = relu(concat(h, msg) @ comm_w[r] + comm_b[r])
    hid = relu(h @ out_w1 + out_b1)
    q   = hid @ out_w2 + out_b2; q[avail==0] = -1e10

Device strategy (pure data parallel, batch split 8 ways):
  * activations feature-major: chunk = [128 part, 512 col] where
    partitions = 2 row-groups x 64 features, columns = 512 rows.  All
    layers use single K=128/M=128 matmuls with block-diagonal weights
    (two row-groups advance in lockstep), so each layer charges one
    512-row pass through the PE instead of four quadrant passes.
  * comm round rewritten as h @ W_self + S @ W_sum with
    W_self = W_h - W_m/31, W_sum = W_m/31, S = per-sample agent sum.
    S comes from an identity-weight matmul whose step-0 (broadcast)
    output AP accumulates the 32 agent columns of each sample into one
    PSUM column; the S @ W_sum term re-broadcasts S via a step-0 rhs AP
    into the same accumulation group as the W_self matmul.
  * 5-stage modulo software pipeline over 64 chunks per core
    (enc | round0 | round1 | out1 | out2), so the PE always has an
    independent chunk's matmuls to run while an evacuation for another
    chunk is in flight.  Evacuations are split Act/DVE to keep both
    below the PE's per-chunk budget.
  * final bias + action mask applied host-side (exact -1e10 lanes);
    the device emits raw hid @ out_w2 in bf16.
  * host pre-packs obs into the feature-major layout and unpacks q
    (layout work is free on host; the device does all the FLOPs).
"""

import contextlib
import sys

import numpy as np

sys.path.insert(0, "/opt/trn_rl_repo")

import ml_dtypes  # noqa: E402

B, N, OBS, H, A, NR = 16384, 32, 64, 64, 16, 2
NCORES = 8
RPC = B * N // NCORES   # rows per core = 65536

CH = 512                # columns per chunk (rows of one row-group)
NCH = RPC // (2 * CH)   # chunks per core = 64 (each chunk = 2 groups x 512)
NSUP = NCH // 2         # obs DMA granularity: [128, 1024] supers
NS_C = CH // N          # samples per row-group per chunk = 16
NQ = NCH // 4           # q output slabs ([128, 512] = 4 chunks each)

_cache = {}


def _build_device_program():
    import concourse.bacc as bacc
    import concourse.mybir as mybir
    from concourse import tile

    F32 = mybir.dt.float32
    BF16 = mybir.dt.bfloat16

    nc = bacc.Bacc("TRN2", target_bir_lowering=False, debug=False)

    obs_d = nc.dram_tensor("obs_pk", [NSUP, 128, 2 * CH], BF16, kind="ExternalInput")
    q_d = nc.dram_tensor("q_pk", [NQ, 128, CH], BF16, kind="ExternalOutput")

    # single packed DMA for all weights / biases (keeps the lead-in short):
    # [Wenc, Wself0, Wsum0, Wself1, Wsum1, W1, idn] at 128 cols each + W2 at 32
    wname = ["Wenc", "Wself0", "Wsum0", "Wself1", "Wsum1", "W1", "idn"]
    w_d = nc.dram_tensor("wpack", [128, 128 * len(wname) + 32], BF16,
                         kind="ExternalInput")
    bname = ["be", "b0", "b1", "bh"]
    b_d = nc.dram_tensor("bpack", [128, len(bname)], F32, kind="ExternalInput")

    Relu = mybir.ActivationFunctionType.Relu
    Copy = mybir.ActivationFunctionType.Copy
    ALU = mybir.AluOpType

    with tile.TileContext(nc) as tc, contextlib.ExitStack() as ctx:
        wp = ctx.enter_context(tc.tile_pool(name="w", bufs=1))
        pool = ctx.enter_context(tc.tile_pool(name="p", bufs=2))
        psum = ctx.enter_context(tc.tile_pool(name="ps", bufs=1, space="PSUM"))

        wt = wp.tile([128, 128 * len(wname) + 32], BF16, tag="wpack", name="wt")
        W = {n: wt[:, 128 * i:128 * (i + 1)] for i, n in enumerate(wname)}
        W["W2"] = wt[:, 128 * len(wname):]
        bt = wp.tile([128, len(bname)], F32, tag="bpack", name="bt")
        BIAS = {n: bt[:, i:i + 1] for i, n in enumerate(bname)}

        obs_t = {}          # super index -> SBUF tile [128, 1024]
        st = {}             # per-chunk state: h0/h1/h2/hid tiles
        qps = {}            # q PSUM slab per 4-chunk block

        def fetch_super(s):
            if s < NSUP and s not in obs_t:
                obs_t[s] = pool.tile([128, 2 * CH], BF16, tag="obs", bufs=4, name="obs_t")
                nc.sync.dma_start(obs_t[s][:], obs_d[s])

        def obs_slice(c):
            t = obs_t[c // 2]
            f = c % 2
            return t[:, f * CH:(f + 1) * CH]

        # --- pipeline stages (chunk c) -------------------------------------
        def stage_S(c, r, h):
            """S (agent-sum) matmul for round r; returns S2 SBUF tile."""
            psS = psum.tile([128, NS_C], F32, tag="S", bufs=2, name="psS")
            rhs = h[:].rearrange("p (S n) -> p n S", n=N)
            outS = psS[:].unsqueeze(1).broadcast_to([128, N, NS_C])
            nc.tensor.matmul(outS, W["idn"], rhs, start=True, stop=True)
            S2 = pool.tile([128, NS_C], BF16, tag=f"S2{r}", bufs=2, name="S2")
            nc.vector.tensor_copy(S2[:], psS[:])
            return S2

        def stage_R(c, r, h, S2):
            """Comm-round linear layer; returns new h tile (evac on Act)."""
            psR = psum.tile([128, CH], F32, tag="stg", bufs=4, name="ps_stg")
            nc.tensor.matmul(psR[:], W[f"Wself{r}"], h[:],
                             start=True, stop=False)
            sb = S2[:].unsqueeze(2).broadcast_to([128, NS_C, N])
            nc.tensor.matmul(psR[:], W[f"Wsum{r}"], sb,
                             start=False, stop=True)
            hn = pool.tile([128, CH], BF16, tag=f"h{1 + r}", bufs=2, name="hn")
            nc.scalar.activation(hn[:], psR[:], Relu, bias=BIAS[f"b{r}"])
            return hn

        def stage0(c):      # encoder
            psE = psum.tile([128, CH], F32, tag="stg", bufs=4, name="ps_stg")
            nc.tensor.matmul(psE[:], W["Wenc"], obs_slice(c),
                             start=True, stop=True)
            h0 = pool.tile([128, CH], BF16, tag="h0", bufs=2, name="h0")
            nc.vector.tensor_scalar(h0[:], psE[:], BIAS["be"], 0.0,
                                    ALU.add, ALU.max)
            st[c] = {"h0": h0}

        def stage1(c):      # round 0 (S first so the copy hides behind E)
            d = st[c]
            S2 = stage_S(c, 0, d["h0"])
            d["S2"] = S2

        def stage1b(c):
            d = st[c]
            d["h1"] = stage_R(c, 0, d["h0"], d.pop("S2"))
            d.pop("h0")

        def stage2(c):      # round 1
            d = st[c]
            S2 = stage_S(c, 1, d["h1"])
            d["S2"] = S2

        def stage2b(c):
            d = st[c]
            d["h2"] = stage_R(c, 1, d["h1"], d.pop("S2"))
            d.pop("h1")

        def stage3(c):      # out1
            d = st[c]
            psH = psum.tile([128, CH], F32, tag="stg", bufs=4, name="ps_stg")
            nc.tensor.matmul(psH[:], W["W1"], d["h2"][:],
                             start=True, stop=True)
            hid = pool.tile([128, CH], BF16, tag="hid", bufs=2, name="hid")
            nc.vector.tensor_scalar(hid[:], psH[:], BIAS["bh"], 0.0,
                                    ALU.add, ALU.max)
            d["hid"] = hid
            d.pop("h2")

        def stage4(c):      # out2 into packed q slab
            d = st.pop(c)
            blk, k = c // 4, c % 4
            if k == 0:
                qps[blk] = psum.tile([128, CH], F32, tag="q", bufs=2, name="psQ")
            psQ = qps[blk]
            nc.tensor.matmul(psQ[32 * k:32 * k + 32, :], W["W2"],
                             d["hid"][:], start=True, stop=True,
                             tile_position=(0, 32 * k), skip_group_check=True)
            if k == 3:
                q_sb = pool.tile([128, CH], BF16, tag="qsb", bufs=2, name="q_sb")
                nc.scalar.activation(q_sb[:], psQ[:], Copy)
                nc.sync.dma_start(q_d[blk], q_sb[:])
                qps.pop(blk)

        # --- modulo schedule ----------------------------------------------
        nc.sync.dma_start(wt[:], w_d[:])
        nc.sync.dma_start(bt[:], b_d[:])
        fetch_super(0)
        fetch_super(1)
        fetch_super(2)
        # PE issue order per step keeps >=2 independent matmuls between an
        # S matmul and the R matmuls that consume its DVE-copied result
        # (also through the drain steps, where stage0 is absent).
        for t in range(NCH + 4):
            if t % 2 == 0:
                fetch_super(t // 2 + 3)
            c1, c2 = t - 1, t - 2
            if 0 <= c1 < NCH:
                stage1(c1)
            if 0 <= c2 < NCH:
                stage2(c2)
            if t < NCH:
                stage0(t)
            if 0 <= c1 < NCH:
                stage1b(c1)
            if 0 <= c2 < NCH:
                stage2b(c2)
            if 0 <= t - 3 < NCH:
                stage3(t - 3)
            if 0 <= t - 4 < NCH:
                stage4(t - 4)

    nc.compile()
    return nc


def _prep_host(obs, enc_w, enc_b, comm_w, comm_b, out_w1, out_b1, out_w2, out_b2,
               available_actions):
    """Build per-core input maps (packed layouts + derived weights)."""
    bf16 = ml_dtypes.bfloat16
    f32 = np.float32

    def bd128(w):  # block-diag duplicate [64,64] -> [128,128]
        o = np.zeros((128, 128), f32)
        o[:64, :64] = w
        o[64:, 64:] = w
        return np.ascontiguousarray(o).astype(bf16)

    def bdq(w):  # block-diag duplicate [64,16] -> [128,32]
        o = np.zeros((128, 32), f32)
        o[:64, :16] = w
        o[64:, 16:] = w
        return np.ascontiguousarray(o).astype(bf16)

    wparts = {"Wenc": bd128(enc_w.astype(f32)), "W1": bd128(out_w1.astype(f32)),
              "W2": bdq(out_w2.astype(f32)),
              "idn": np.eye(128, dtype=f32).astype(bf16)}
    for r in range(NR):
        wh = comm_w[r][:H].astype(f32)
        wm = comm_w[r][H:].astype(f32) / (N - 1)
        wparts[f"Wself{r}"] = bd128(wh - wm)
        wparts[f"Wsum{r}"] = bd128(wm)
    worder = ["Wenc", "Wself0", "Wsum0", "Wself1", "Wsum1", "W1", "idn", "W2"]
    wpack = np.ascontiguousarray(
        np.concatenate([wparts[n] for n in worder], axis=1))
    bpack = np.ascontiguousarray(
        np.stack([np.concatenate([v, v]) for v in
                  (enc_b, comm_b[0], comm_b[1], out_b1)], axis=1).astype(f32))
    weights = {"wpack": wpack}
    biases = {"bpack": bpack}

    rows = np.ascontiguousarray(obs.reshape(B * N, OBS))

    in_maps = []
    for c in range(NCORES):
        ro = rows[c * RPC:(c + 1) * RPC]
        # [NSUP, phalf, fhalf, row, feat] -> [NSUP, phalf*feat, fhalf*row]
        opk = ro.reshape(NSUP, 2, 2, CH, OBS).transpose(0, 1, 4, 2, 3) \
                .reshape(NSUP, 128, 2 * CH).astype(bf16)
        m = {"obs_pk": np.ascontiguousarray(opk)}
        m.update(weights)
        m.update(biases)
        in_maps.append(m)
    return in_maps


def _unpack_output(results, out_b2, available_actions):
    f32 = np.float32
    qs = []
    for r in results:
        qpk = np.asarray(r["q_pk"]).astype(f32)        # [NQ, 128, CH]
        # partitions = (k=chunk%4, p, a); chunk c = 2s + f
        q = qpk.reshape(NQ, 4, 2, A, CH).transpose(0, 1, 2, 4, 3) \
               .reshape(NSUP, 2, 2, CH, A).transpose(0, 2, 1, 3, 4) \
               .reshape(RPC, A)                         # rows: [s, p, f, row]
        qs.append(q)
    q = np.concatenate(qs, axis=0).reshape(B, N, A)
    q = q + out_b2.astype(f32)[None, None, :]
    return np.where(available_actions == 0, f32(-1e10), q)


def run_on_device(in_maps, trace=False):
    from concourse.bass_utils import run_bass_kernel_spmd

    if "nc" not in _cache:
        _cache["nc"] = _build_device_program()
    return run_bass_kernel_spmd(_cache["nc"], in_maps,
                                core_ids=list(range(NCORES)), trace=trace)


def kernel(obs, enc_w, enc_b, comm_w, comm_b, out_w1, out_b1, out_w2, out_b2,
           available_actions):
    args = [np.asarray(x) for x in
            (obs, enc_w, enc_b, comm_w, comm_b, out_w1, out_b1, out_w2, out_b2,
             available_actions)]
    in_maps = _prep_host(*args)
    res = run_on_device(in_maps)
    return _unpack_output(res.results, np.asarray(out_b2),
                          np.asarray(available_actions))


# revision 16
# speedup vs baseline: 3.5688x; 1.1423x over previous
"""CommNet forward kernel for 8 Trainium2 NeuronCores.

Reference computation (per sample of N=32 agents, batch B=16384):
    h   = relu(obs @ enc_w + enc_b)                    # [B,N,64]
    2x:  msg = (sum_n h - h)/31
         h   = relu(concat(h, msg) @ comm_w[r] + comm_b[r])
    hid = relu(h @ out_w1 + out_b1)
    q   = hid @ out_w2 + out_b2; q[avail==0] = -1e10

Device strategy (pure data parallel, batch split 8 ways):
  * activations feature-major: chunk = [128 part, 512 col] where
    partitions = 2 row-groups x 64 features, columns = 512 rows.  All
    layers use single K=128/M=128 matmuls with block-diagonal weights
    (two row-groups advance in lockstep), so each layer charges one
    512-row pass through the PE instead of four quadrant passes.
  * comm round rewritten as h @ W_self + S @ W_sum with
    W_self = W_h - W_m/31, W_sum = W_m/31, S = per-sample agent sum.
    S comes from an identity-weight matmul whose step-0 (broadcast)
    output AP accumulates the 32 agent columns of each sample into one
    PSUM column.  Round 1 re-broadcasts S via a step-0 rhs AP into the
    same accumulation group as its W_self matmul.
  * round 0 runs as a single fp8e4m3 DoubleRow matmul (2 K-tiles at
    0.5 cycles/row): k-tile 0 = fp8 h0, k-tile 1 = a dense per-sample
    S broadcast materialized by the otherwise-idle GPSIMD engine from
    the S PSUM column.  Only this layer is fp8; everything downstream
    stays bf16.
  * 7-stage modulo software pipeline over 64 chunks per core
    (enc | S0+broadcast | round0 | S1 | round1 | out1 | out2), so the
    PE always has independent matmuls to run while evacuations and the
    GPSIMD broadcast for other chunks are in flight.  Evacuations are
    split across Act/DVE/GPSIMD to keep each below the PE budget.
  * final bias + action mask applied host-side (exact -1e10 lanes);
    the device emits raw hid @ out_w2 in bf16.
  * host pre-packs obs into the feature-major layout and unpacks q
    (layout work is free on host; the device does all the FLOPs).
"""

import contextlib
import sys

import numpy as np

sys.path.insert(0, "/opt/trn_rl_repo")

import ml_dtypes  # noqa: E402

B, N, OBS, H, A, NR = 16384, 32, 64, 64, 16, 2
NCORES = 8
RPC = B * N // NCORES   # rows per core = 65536

CH = 512                # columns per chunk (rows of one row-group)
NCH = RPC // (2 * CH)   # chunks per core = 64 (each chunk = 2 groups x 512)
NSUP = NCH // 2         # obs DMA granularity: [128, 1024] supers
NS_C = CH // N          # samples per row-group per chunk = 16
NQ = NCH // 4           # q output slabs ([128, 512] = 4 chunks each)

_cache = {}


def _build_device_program():
    import concourse.bacc as bacc
    import concourse.mybir as mybir
    from concourse import tile

    F32 = mybir.dt.float32
    BF16 = mybir.dt.bfloat16
    FP8 = mybir.dt.float8e4

    nc = bacc.Bacc("TRN2", target_bir_lowering=False, debug=False)

    obs_d = nc.dram_tensor("obs_pk", [NSUP, 128, 2 * CH], BF16, kind="ExternalInput")
    q_d = nc.dram_tensor("q_pk", [NQ, 128, CH], BF16, kind="ExternalOutput")

    # single packed DMA for all weights / biases (keeps the lead-in short):
    # [Wenc, Wself0, Wsum0, Wself1, Wsum1, W1, idn] at 128 cols each + W2 at 32
    wname = ["Wenc", "Wself1", "Wsum1", "W1", "idn"]
    w_d = nc.dram_tensor("wpack", [128, 128 * len(wname) + 32], BF16,
                         kind="ExternalInput")
    w8_d = nc.dram_tensor("w8pack", [128, 384], FP8, kind="ExternalInput")
    bname = ["be", "b0", "b1", "bh"]
    b_d = nc.dram_tensor("bpack", [128, len(bname)], F32, kind="ExternalInput")

    Relu = mybir.ActivationFunctionType.Relu
    Copy = mybir.ActivationFunctionType.Copy
    ALU = mybir.AluOpType

    with tile.TileContext(nc) as tc, contextlib.ExitStack() as ctx:
        wp = ctx.enter_context(tc.tile_pool(name="w", bufs=1))
        pool = ctx.enter_context(tc.tile_pool(name="p", bufs=2))
        psum = ctx.enter_context(tc.tile_pool(name="ps", bufs=1, space="PSUM"))

        wt = wp.tile([128, 128 * len(wname) + 32], BF16, tag="wpack", name="wt")
        W = {n: wt[:, 128 * i:128 * (i + 1)] for i, n in enumerate(wname)}
        W["W2"] = wt[:, 128 * len(wname):]
        bt = wp.tile([128, len(bname)], F32, tag="bpack", name="bt")
        BIAS = {n: bt[:, i:i + 1] for i, n in enumerate(bname)}
        w8t = wp.tile([128, 384], FP8, tag="w8pack", name="w8t")
        WDR = w8t[:, 0:256].rearrange("p (t m) -> p t m", t=2)
        IDN8 = w8t[:, 256:384]

        obs_t = {}          # super index -> SBUF tile [128, 1024]
        st = {}             # per-chunk state: h0/h1/h2/hid tiles
        qps = {}            # q PSUM slab per 4-chunk block

        def fetch_super(s):
            if s < NSUP and s not in obs_t:
                obs_t[s] = pool.tile([128, 2 * CH], BF16, tag="obs", bufs=4, name="obs_t")
                nc.sync.dma_start(obs_t[s][:], obs_d[s])

        def obs_slice(c):
            t = obs_t[c // 2]
            f = c % 2
            return t[:, f * CH:(f + 1) * CH]

        # --- pipeline stages (chunk c) ---
        def stage0(c):      # encoder -> fp8 h0 in k-tile 0 of hS
            psE = psum.tile([128, CH], F32, tag="stg", bufs=4, name="ps_stg")
            nc.tensor.matmul(psE[:], W["Wenc"], obs_slice(c),
                             start=True, stop=True)
            hS = pool.tile([128, 2 * CH], FP8, tag="h0", bufs=3, name="hS")
            nc.vector.tensor_scalar(hS[:, 0:CH], psE[:], BIAS["be"], 0.0,
                                    ALU.add, ALU.max)
            st[c] = {"hS": hS}

        def stage1(c):      # round 0: S matmul + dense S broadcast (Pool)
            d = st[c]
            hS = d["hS"]
            psS = psum.tile([128, NS_C], F32, tag="S", bufs=2, name="psS")
            rhs = hS[:, 0:CH].rearrange("p (S n) -> p n S", n=N)
            outS = psS[:].unsqueeze(1).broadcast_to([128, N, NS_C])
            nc.tensor.matmul(outS, IDN8, rhs, start=True, stop=True)
            S2 = pool.tile([128, NS_C], FP8, tag="S20", bufs=2, name="S2")
            nc.vector.tensor_copy(S2[:], psS[:])
            sbo = hS[:, CH:2 * CH].rearrange("p (S n) -> p S n", n=N)
            sbi = S2[:].unsqueeze(2).broadcast_to([128, NS_C, N])
            nc.gpsimd.tensor_copy(sbo, sbi)

        def stage1b(c):     # round 0 linear: one fp8 DoubleRow matmul
            d = st[c]
            hS = d.pop("hS")
            psR = psum.tile([128, CH], F32, tag="stg", bufs=4, name="ps_stg")
            nc.tensor.matmul(psR[:], WDR,
                             hS[:].rearrange("p (t x) -> p t x", t=2),
                             start=True, stop=True,
                             perf_mode=mybir.MatmulPerfMode.DoubleRow)
            h1 = pool.tile([128, CH], BF16, tag="h1", bufs=3, name="h1")
            nc.scalar.activation(h1[:], psR[:], Relu, bias=BIAS["b0"])
            d["h1"] = h1

        def stage2(c):      # round 1
            d = st[c]
            psS = psum.tile([128, NS_C], F32, tag="S", bufs=2, name="psS")
            rhs = d["h1"][:].rearrange("p (S n) -> p n S", n=N)
            outS = psS[:].unsqueeze(1).broadcast_to([128, N, NS_C])
            nc.tensor.matmul(outS, W["idn"], rhs, start=True, stop=True)
            S2 = pool.tile([128, NS_C], BF16, tag="S21", bufs=2, name="S2")
            nc.scalar.copy(S2[:], psS[:])
            d["S2"] = S2

        def stage2b(c):
            d = st[c]
            psR = psum.tile([128, CH], F32, tag="stg", bufs=4, name="ps_stg")
            nc.tensor.matmul(psR[:], W["Wsum1"],
                             d.pop("S2")[:].unsqueeze(2)
                             .broadcast_to([128, NS_C, N]),
                             start=True, stop=False)
            nc.tensor.matmul(psR[:], W["Wself1"], d.pop("h1")[:],
                             start=False, stop=True)
            h2 = pool.tile([128, CH], BF16, tag="h2", bufs=2, name="h2")
            nc.scalar.activation(h2[:], psR[:], Relu, bias=BIAS["b1"])
            d["h2"] = h2

        def stage3(c):      # out1
            d = st[c]
            psH = psum.tile([128, CH], F32, tag="stg", bufs=4, name="ps_stg")
            nc.tensor.matmul(psH[:], W["W1"], d["h2"][:],
                             start=True, stop=True)
            hid = pool.tile([128, CH], BF16, tag="hid", bufs=2, name="hid")
            nc.vector.tensor_scalar(hid[:], psH[:], BIAS["bh"], 0.0,
                                    ALU.add, ALU.max)
            d["hid"] = hid
            d.pop("h2")

        def stage4(c):      # out2 into packed q slab
            d = st.pop(c)
            blk, k = c // 4, c % 4
            if k == 0:
                qps[blk] = psum.tile([128, CH], F32, tag="q", bufs=2, name="psQ")
            psQ = qps[blk]
            nc.tensor.matmul(psQ[32 * k:32 * k + 32, :], W["W2"],
                             d["hid"][:], start=True, stop=True,
                             tile_position=(0, 32 * k), skip_group_check=True)
            if k == 3:
                q_sb = pool.tile([128, CH], BF16, tag="qsb", bufs=2, name="q_sb")
                nc.scalar.activation(q_sb[:], psQ[:], Copy)
                nc.sync.dma_start(q_d[blk], q_sb[:])
                qps.pop(blk)

        # --- modulo schedule ----------------------------------------------
        nc.sync.dma_start(wt[:], w_d[:])
        nc.sync.dma_start(bt[:], b_d[:])
        nc.sync.dma_start(w8t[:], w8_d[:])
        fetch_super(0)
        fetch_super(1)
        fetch_super(2)
        # PE issue order per step keeps >=2 independent matmuls between an
        # S matmul and the R matmuls that consume its DVE-copied result
        # (also through the drain steps, where stage0 is absent).
        for t in range(NCH + 6):
            if t % 2 == 0:
                fetch_super(t // 2 + 3)
            if 0 <= t - 1 < NCH:
                stage1(t - 1)
            if 0 <= t - 3 < NCH:
                stage2(t - 3)
            if t < NCH:
                stage0(t)
            if 0 <= t - 2 < NCH:
                stage1b(t - 2)
            if 0 <= t - 4 < NCH:
                stage2b(t - 4)
            if 0 <= t - 5 < NCH:
                stage3(t - 5)
            if 0 <= t - 6 < NCH:
                stage4(t - 6)

    nc.compile()
    return nc


def _prep_host(obs, enc_w, enc_b, comm_w, comm_b, out_w1, out_b1, out_w2, out_b2,
               available_actions):
    """Build per-core input maps (packed layouts + derived weights)."""
    bf16 = ml_dtypes.bfloat16
    f32 = np.float32

    def bd128(w):  # block-diag duplicate [64,64] -> [128,128]
        o = np.zeros((128, 128), f32)
        o[:64, :64] = w
        o[64:, 64:] = w
        return np.ascontiguousarray(o).astype(bf16)

    def bdq(w):  # block-diag duplicate [64,16] -> [128,32]
        o = np.zeros((128, 32), f32)
        o[:64, :16] = w
        o[64:, 16:] = w
        return np.ascontiguousarray(o).astype(bf16)

    wparts = {"Wenc": bd128(enc_w.astype(f32)), "W1": bd128(out_w1.astype(f32)),
              "W2": bdq(out_w2.astype(f32)),
              "idn": np.eye(128, dtype=f32).astype(bf16)}
    for r in range(NR):
        wh = comm_w[r][:H].astype(f32)
        wm = comm_w[r][H:].astype(f32) / (N - 1)
        wparts[f"Wself{r}"] = bd128(wh - wm)
        wparts[f"Wsum{r}"] = bd128(wm)
    worder = ["Wenc", "Wself1", "Wsum1", "W1", "idn", "W2"]
    wpack = np.ascontiguousarray(
        np.concatenate([wparts[n] for n in worder], axis=1))
    f8 = ml_dtypes.float8_e4m3
    w8pack = np.ascontiguousarray(np.concatenate(
        [wparts["Wself0"].astype(np.float32), wparts["Wsum0"].astype(np.float32),
         np.eye(128, dtype=np.float32)], axis=1)).astype(f8)
    bpack = np.ascontiguousarray(
        np.stack([np.concatenate([v, v]) for v in
                  (enc_b, comm_b[0], comm_b[1], out_b1)], axis=1).astype(f32))
    weights = {"wpack": wpack, "w8pack": w8pack}
    biases = {"bpack": bpack}

    rows = np.ascontiguousarray(obs.reshape(B * N, OBS))

    in_maps = []
    for c in range(NCORES):
        ro = rows[c * RPC:(c + 1) * RPC]
        # [NSUP, phalf, fhalf, row, feat] -> [NSUP, phalf*feat, fhalf*row]
        opk = ro.reshape(NSUP, 2, 2, CH, OBS).transpose(0, 1, 4, 2, 3) \
                .reshape(NSUP, 128, 2 * CH).astype(bf16)
        m = {"obs_pk": np.ascontiguousarray(opk)}
        m.update(weights)
        m.update(biases)
        in_maps.append(m)
    return in_maps


def _unpack_output(results, out_b2, available_actions):
    f32 = np.float32
    qs = []
    for r in results:
        qpk = np.asarray(r["q_pk"]).astype(f32)        # [NQ, 128, CH]
        # partitions = (k=chunk%4, p, a); chunk c = 2s + f
        q = qpk.reshape(NQ, 4, 2, A, CH).transpose(0, 1, 2, 4, 3) \
               .reshape(NSUP, 2, 2, CH, A).transpose(0, 2, 1, 3, 4) \
               .reshape(RPC, A)                         # rows: [s, p, f, row]
        qs.append(q)
    q = np.concatenate(qs, axis=0).reshape(B, N, A)
    q = q + out_b2.astype(f32)[None, None, :]
    return np.where(available_actions == 0, f32(-1e10), q)


def run_on_device(in_maps, trace=False):
    from concourse.bass_utils import run_bass_kernel_spmd

    if "nc" not in _cache:
        _cache["nc"] = _build_device_program()
    return run_bass_kernel_spmd(_cache["nc"], in_maps,
                                core_ids=list(range(NCORES)), trace=trace)


def kernel(obs, enc_w, enc_b, comm_w, comm_b, out_w1, out_b1, out_w2, out_b2,
           available_actions):
    args = [np.asarray(x) for x in
            (obs, enc_w, enc_b, comm_w, comm_b, out_w1, out_b1, out_w2, out_b2,
             available_actions)]
    in_maps = _prep_host(*args)
    res = run_on_device(in_maps)
    return _unpack_output(res.results, np.asarray(out_b2),
                          np.asarray(available_actions))
